# revision 1
# baseline (speedup 1.0000x reference)
"""Trainium2 Bass kernel for nn_EpsiLayer: per-channel causal full-length
time convolution  out[b,t,j] = P[b,t,j] + sum_{k<=t} g[k,j] * P[b,t-k,j].

Identity fold: with g'[0] = g[0] + 1, out = causal_conv(g', P) exactly.

Per channel j the conv is a lower-triangular Toeplitz (T x T) matmul.
Blocked into C=128 chunks: y_i += W_d @ x_{i-d},
W_d[b, a] = g'[d*128 + a - b] (zero for negative index).

On-device scheme per channel:
  - One sliding-window DMA materializes all 32 Toeplitz weight tiles:
    W[p, d*128 + a] = gpad[d*128 + a + p], gpad = 127 zeros ++ g' (bf16).
    (Stationary partition p corresponds to within-block source time
    b = 127 - p; the moving operand is time-reversed within each block
    on the host so the contraction pairs correctly.)
  - 32 accumulating matmuls into one PSUM tile [128, 256]:
    matmul d: lhsT = W[:, d*128:(d+1)*128], rhs = xmov columns of source
    blocks 0..31-d, out = PSUM columns 8d:256 (output blocks d..31).
  - Copy PSUM -> SBUF -> DRAM.

Sharding: channel-parallel, NR=256 -> 32 channels per core, no comms.
"""

import sys
import numpy as np

try:
    from concourse import bacc, tile  # noqa: F401
except ImportError:  # grading env may not have it on sys.path yet
    sys.path.insert(0, "/opt/trn_rl_repo")

import ml_dtypes

B, T, NR = 8, 4096, 256
C = 128
NB = T // C            # 32 time blocks
N_CORES = 8
CPC = NR // N_CORES    # 32 channels per core
COLS = CPC * NB * B    # 8192 columns per core
GLEN = 127 + T + 1     # 4224

_cache = {}


def _build_nc():
    from concourse import bacc, tile
    import concourse.mybir as mybir
    from concourse.bass_types import AP

    nc = bacc.Bacc("TRN2", target_bir_lowering=False, debug=False)

    g_d = nc.dram_tensor("gpads", [CPC, GLEN], mybir.dt.bfloat16,
                         kind="ExternalInput")
    x_d = nc.dram_tensor("xmov", [C, COLS], mybir.dt.bfloat16,
                         kind="ExternalInput")
    o_d = nc.dram_tensor("out", [C, COLS], mybir.dt.float32,
                         kind="ExternalOutput")

    g_handle = g_d.ap().tensor

    with tile.TileContext(nc) as tc:
        with (
            tc.tile_pool(name="xpool", bufs=1) as xpool,
            tc.tile_pool(name="wpool", bufs=3) as wpool,
            tc.tile_pool(name="opool", bufs=4) as opool,
            tc.tile_pool(name="psum", bufs=4, space="PSUM") as psum,
        ):
            xmov = xpool.tile([C, COLS], mybir.dt.bfloat16)
            nc.sync.dma_start(xmov[:], x_d[:])

            for j in range(CPC):
                wj = wpool.tile([C, NB * C], mybir.dt.bfloat16)
                diag = AP(tensor=g_handle, offset=j * GLEN,
                          ap=[[1, C], [1, NB * C]])
                nc.sync.dma_start(wj[:], diag)

                acc = psum.tile([C, NB * B], mybir.dt.float32)
                xj = xmov[:, j * NB * B:(j + 1) * NB * B]
                for d in range(NB):
                    ncols = B * (NB - d)
                    nc.tensor.matmul(
                        acc[:, d * B:],
                        wj[:, d * C:(d + 1) * C],
                        xj[:, :ncols],
                        start=(d == 0),
                        stop=(d == NB - 1),
                    )

                oj = opool.tile([C, NB * B], mybir.dt.float32)
                nc.any.tensor_copy(oj[:], acc[:])
                nc.scalar.dma_start(o_d[:, j * NB * B:(j + 1) * NB * B], oj[:])

    nc.compile()
    return nc


def _prep_inputs(P, g):
    """Host-side shard + layout prep. Returns in_maps list for 8 cores."""
    bf16 = ml_dtypes.bfloat16
    P = np.asarray(P)
    g = np.asarray(g)

    gmod = g.astype(np.float32).copy()
    gmod[0, :] += 1.0

    in_maps = []
    for core in range(N_CORES):
        lo, hi = core * CPC, (core + 1) * CPC
        gpads = np.zeros((CPC, GLEN), dtype=bf16)
        gpads[:, 127:127 + T] = gmod[:, lo:hi].T.astype(bf16)

        Pc = P[:, :, lo:hi]                                  # (B, T, CPC)
        x4 = Pc.reshape(B, NB, C, CPC)                       # (b, i, c, j)
        xmov = np.ascontiguousarray(
            x4[:, :, ::-1, :].transpose(2, 3, 1, 0)          # (p, j, i, b)
        ).reshape(C, COLS).astype(bf16)

        in_maps.append({"gpads": gpads, "xmov": xmov})
    return in_maps


def _unshard(results):
    out = np.empty((B, T, NR), np.float32)
    for core in range(N_CORES):
        oc = np.asarray(results[core]["out"], dtype=np.float32)
        oc = oc.reshape(C, CPC, NB, B).transpose(3, 2, 0, 1)  # (b, i, a, j)
        out[:, :, core * CPC:(core + 1) * CPC] = oc.reshape(B, T, CPC)
    return out


def kernel(P, g):
    from concourse.bass_utils import run_bass_kernel_spmd

    if "nc" not in _cache:
        _cache["nc"] = _build_nc()
    nc = _cache["nc"]

    in_maps = _prep_inputs(P, g)
    res = run_bass_kernel_spmd(nc, in_maps, list(range(N_CORES)))
    return _unshard(res.results)


if __name__ == "__main__":
    rng = np.random.default_rng(0)
    P = rng.standard_normal((B, T, NR)).astype(np.float32)
    g = (rng.standard_normal((T, NR)) * 0.1).astype(np.float32)
    out = kernel(P, g)
    print("out shape:", out.shape, out.dtype)


# revision 15
# speedup vs baseline: 10942.9425x; 10942.9425x over previous
"""Trainium2 Bass kernel for nn_EpsiLayer: per-channel causal full-length
time convolution  out[b,t,j] = P[b,t,j] + sum_{k<=t} g[k,j] * P[b,t-k,j].

Identity fold: with g'[0] = g[0] + 1, out = causal_conv(g', P) exactly.

Per channel j the conv is a lower-triangular Toeplitz (T x T) matmul.
Blocked into C=128 chunks: y_i += W_d @ x_{i-d},
W_d[b, a] = g'[d*128 + a - b] (zero for negative index).

On-device scheme per channel:
  - One sliding-window DMA materializes all 32 Toeplitz weight tiles:
    W[p, d*128 + a] = gpad[d*128 + a + p], gpad = 127 zeros ++ g' (bf16).
    (Stationary partition p corresponds to within-block source time
    b = 127 - p; the moving operand is time-reversed within each block
    on the host so the contraction pairs correctly.)
  - 32 accumulating matmuls into one PSUM tile [128, 256]:
    matmul d: lhsT = W[:, d*128:(d+1)*128], rhs = xmov columns of source
    blocks 0..31-d, out = PSUM columns 8d:256 (output blocks d..31).
  - Copy PSUM -> SBUF -> DRAM.

Sharding: channel-parallel, NR=256 -> 32 channels per core, no comms.
"""

import sys
import numpy as np

try:
    from concourse import bacc, tile  # noqa: F401
except ImportError:  # grading env may not have it on sys.path yet
    sys.path.insert(0, "/opt/trn_rl_repo")

import ml_dtypes

B, T, NR = 8, 4096, 256
C = 128
NB = T // C            # 32 time blocks
N_CORES = 8
CPC = NR // N_CORES    # 32 channels per core
COLS = CPC * NB * B    # 8192 columns per core
GLEN = 127 + T + 1     # 4224

_cache = {}


def _build_nc(reps=1, OB=1, XSPLIT=4, walt=False, oeng="scalar", wbufs=3,
              pbufs=4, obufs=4, ceng="vector", wsrc="dense", obf16=False):
    from concourse import bacc, tile
    import concourse.mybir as mybir
    from concourse.bass_types import AP

    nc = bacc.Bacc("TRN2", target_bir_lowering=False, debug=False)

    if wsrc in ("dense", "mix"):
        nden = CPC if wsrc == "dense" else (CPC + 1) // 2
        w_d = nc.dram_tensor("wdense", [nden, C, NB * C], mybir.dt.bfloat16,
                             kind="ExternalInput")
    if wsrc in ("diag", "mix"):
        g_d = nc.dram_tensor("gpads", [CPC, GLEN], mybir.dt.bfloat16,
                             kind="ExternalInput")
        g_handle = g_d.ap().tensor
    x_d = nc.dram_tensor("xmov", [C, COLS], mybir.dt.bfloat16,
                         kind="ExternalInput")
    odt = mybir.dt.bfloat16 if obf16 else mybir.dt.float32
    o_d = nc.dram_tensor("out", [C, COLS], odt, kind="ExternalOutput")

    with tile.TileContext(nc) as tc:
        with (
            tc.tile_pool(name="xpool", bufs=1) as xpool,
            tc.tile_pool(name="wpool", bufs=wbufs) as wpool,
            tc.tile_pool(name="opool", bufs=obufs) as opool,
            tc.tile_pool(name="psum", bufs=pbufs, space="PSUM") as psum,
        ):
            xmov = xpool.tile([C, COLS], mybir.dt.bfloat16)
            # split the input load so channel-0 matmuls start sooner
            XCH = COLS // XSPLIT
            for k in range(XSPLIT):
                nc.scalar.dma_start(xmov[:, k * XCH:(k + 1) * XCH],
                                    x_d[:, k * XCH:(k + 1) * XCH])

            def body(_iv=None):
                for j in range(CPC):
                    wj = wpool.tile([C, NB * C], mybir.dt.bfloat16)
                    weng = nc.scalar if (walt and j % 2) else nc.sync
                    use_dense = (wsrc == "dense") or (wsrc == "mix" and j % 2 == 0)
                    if use_dense:
                        widx = j if wsrc == "dense" else j // 2
                        weng.dma_start(wj[:], w_d[widx])
                    else:
                        diag = AP(tensor=g_handle, offset=j * GLEN,
                                  ap=[[1, C], [1, NB * C]])
                        weng.dma_start(wj[:], diag)

                    acc = psum.tile([C, NB * B], mybir.dt.float32)
                    xj = xmov[:, j * NB * B:(j + 1) * NB * B]
                    for d in range(NB):
                        ncols = B * (NB - d)
                        nc.tensor.matmul(
                            acc[:, d * B:],
                            wj[:, d * C:(d + 1) * C],
                            xj[:, :ncols],
                            start=(d == 0),
                            stop=(d == NB - 1),
                        )

                    if j % OB == 0:
                        og = opool.tile([C, OB * NB * B], odt, tag="og")
                    getattr(nc, ceng).tensor_copy(
                        og[:, (j % OB) * NB * B:(j % OB + 1) * NB * B], acc[:])
                    if j % OB == OB - 1:
                        j0 = j - (OB - 1)
                        getattr(nc, oeng).dma_start(
                            o_d[:, j0 * NB * B:(j0 + OB) * NB * B], og[:])

            if reps == 1:
                body()
            else:
                with tc.For_i(0, reps, 1) as iv:
                    body(iv)

    nc.compile()
    return nc


def _prep_inputs(P, g, wsrc="dense"):
    """Host-side shard + layout prep. Returns in_maps list for 8 cores."""
    bf16 = ml_dtypes.bfloat16
    P = np.asarray(P)
    g = np.asarray(g)

    gmod = g.astype(np.float32).copy()
    gmod[0, :] += 1.0

    in_maps = []
    for core in range(N_CORES):
        lo, hi = core * CPC, (core + 1) * CPC
        gpads = np.zeros((CPC, GLEN), dtype=bf16)
        gpads[:, 127:127 + T] = gmod[:, lo:hi].T.astype(bf16)
        # Dense Toeplitz expansion: wdense[j, p, e] = gpads[j, e + p].
        # (Device-side sliding-window DMA hits HBM channel conflicts —
        #  all engines read the same small region — so expand on host.)
        Pc = P[:, :, lo:hi]                                  # (B, T, CPC)
        x4 = Pc.reshape(B, NB, C, CPC)                       # (b, i, c, j)
        xmov = np.ascontiguousarray(
            x4[:, :, ::-1, :].transpose(2, 3, 1, 0)          # (p, j, i, b)
        ).reshape(C, COLS).astype(bf16)

        m = {"xmov": xmov}
        if wsrc in ("dense", "mix"):
            src = gpads if wsrc == "dense" else gpads[0::2]
            sw = np.lib.stride_tricks.sliding_window_view(
                src, NB * C, axis=1)
            m["wdense"] = np.ascontiguousarray(sw[:, :C, :])
        if wsrc in ("diag", "mix"):
            m["gpads"] = gpads
        in_maps.append(m)
    return in_maps


def _unshard(results):
    out = np.empty((B, T, NR), np.float32)
    for core in range(N_CORES):
        oc = np.asarray(results[core]["out"], dtype=np.float32)
        oc = oc.reshape(C, CPC, NB, B).transpose(3, 2, 0, 1)  # (b, i, a, j)
        out[:, :, core * CPC:(core + 1) * CPC] = oc.reshape(B, T, CPC)
    return out


def kernel(P, g):
    from concourse.bass_utils import run_bass_kernel_spmd

    if "nc" not in _cache:
        _cache["nc"] = _build_nc()
    nc = _cache["nc"]

    in_maps = _prep_inputs(P, g)
    res = run_bass_kernel_spmd(nc, in_maps, list(range(N_CORES)))
    return _unshard(res.results)


if __name__ == "__main__":
    rng = np.random.default_rng(0)
    P = rng.standard_normal((B, T, NR)).astype(np.float32)
    g = (rng.standard_normal((T, NR)) * 0.1).astype(np.float32)
    out = kernel(P, g)
    print("out shape:", out.shape, out.dtype)
